# revision 9
# baseline (speedup 1.0000x reference)
"""Qwen3-style GQA attention (B=1, S=2048, DM=2048, H=16, KV=4, D=128) on 8 TRN2 cores.

Sharding: tensor-parallel over heads. Core c computes Q heads {2c, 2c+1} and
KV head c//2 end-to-end, then a partial output hs_part = gated_local @ Wo_rows.
Host sums the 8 partials.

All matmuls run in float32r (fp32 with 11-bit-rounded mantissa operands, exact
products, fp32 PSUM accumulation) at full PE rate. Host pre-rounds constant
inputs; on-chip producers write float32r tiles directly.

Layouts: everything "transposed" [feature, seq]: projections consume hsT with
the contraction dim (model dim) on partitions, producing QT/KT/VT/GateT
naturally. Scores are computed per q-block as [q:128 x k] so softmax max/exp
run along the free dim (ACT per-partition bias + accum_out for Z). P^T for the
AV matmul comes from a per-(q-block, k-block) matmul against diag(1/Z), which
fuses the softmax normalization into the transpose.
"""

import numpy as np

S = 2048
DM = 2048
D = 128
HPC = 2           # q heads per core
NCORES = 8
SCALING = float(D) ** 0.5
EPS = 1e-6
P = 128
KCH = DM // P     # 16 contraction chunks for projections
NQB = S // P      # 16 q blocks
NSC = S // 512    # 4 seq chunks of 512

_cache = {}


def _round_fp32r(x):
    x = np.ascontiguousarray(x, dtype=np.float32)
    b = x.view(np.uint32).astype(np.uint64)
    lsb = (b >> 12) & 1
    r = (b + 0x7FF + lsb) & 0xFFFFF000
    return r.astype(np.uint32).view(np.float32)


def _build_nc():
    import concourse.tile as tile
    from concourse import bacc, mybir

    F32 = mybir.dt.float32
    F32R = mybir.dt.float32r
    AF = mybir.ActivationFunctionType
    from concourse.alu_op_type import AluOpType as ALU
    AX = mybir.AxisListType.X

    nc = bacc.Bacc(None, target_bir_lowering=False, debug=False)

    with nc.allow_low_precision(reason="fp32r operands are a deliberate "
                                "precision/speed tradeoff (~1e-4 rel)"), \
         tile.TileContext(nc) as tc:
        with tc.tile_pool(name="dram", bufs=1, space="DRAM") as dram:
            hsT = dram.tile([DM, S], F32R, kind="ExternalInput", name="hsT", uniquify=False)
            wq = dram.tile([DM, HPC * P], F32R, kind="ExternalInput", name="wq", uniquify=False)
            wg = dram.tile([DM, HPC * P], F32R, kind="ExternalInput", name="wg", uniquify=False)
            wk = dram.tile([DM, P], F32R, kind="ExternalInput", name="wk", uniquify=False)
            wv = dram.tile([DM, P], F32R, kind="ExternalInput", name="wv", uniquify=False)
            wo = dram.tile([HPC * P, DM], F32R, kind="ExternalInput", name="wo", uniquify=False)
            cosT = dram.tile([P, S], F32, kind="ExternalInput", name="cosT", uniquify=False)
            sinT = dram.tile([P, S], F32, kind="ExternalInput", name="sinT", uniquify=False)
            qw = dram.tile([P, 1], F32, kind="ExternalInput", name="qw", uniquify=False)
            kw = dram.tile([P, 1], F32, kind="ExternalInput", name="kw", uniquify=False)
            rt = dram.tile([P, P], F32R, kind="ExternalInput", name="rt", uniquify=False)
            ident = dram.tile([P, P], F32R, kind="ExternalInput", name="ident", uniquify=False)
            onec = dram.tile([P, 1], F32R, kind="ExternalInput", name="onec", uniquify=False)
            oner = dram.tile([1, P], F32R, kind="ExternalInput", name="oner", uniquify=False)
            triu = dram.tile([P, P], F32, kind="ExternalInput", name="triu", uniquify=False)
            out = dram.tile([S, DM], F32, kind="ExternalOutput", name="out", uniquify=False)

        # persistent SBUF (whole kernel): ~82.5 KB/partition
        with tc.tile_pool(name="persist", bufs=1) as pers:
            cosT_sb = pers.tile([P, S], F32)
            sinT_sb = pers.tile([P, S], F32)
            qw_sb = pers.tile([P, 1], F32)
            kw_sb = pers.tile([P, 1], F32)
            rt_sb = pers.tile([P, P], F32R)
            ident_sb = pers.tile([P, P], F32R)
            onec_sb = pers.tile([P, 1], F32R)
            oner_sb = pers.tile([1, P], F32R)
            triu_sb = pers.tile([P, P], F32)
            eps_sb = pers.tile([1, 1], F32)
            khat = pers.tile([P, S], F32R)          # normed+roped K^T
            v_r = pers.tile([P, NQB, P], F32R)      # V untransposed (s-major blocks)
            gt_raw = pers.tile([P, HPC, S], F32)    # raw Gate^T per head
            gated_r = pers.tile([P, HPC, S], F32R)  # sigmoid(g) * O^T
            qhat = pers.tile([P, S], F32R)          # normed+roped Q^T (per-head reuse)
            ot_sb = pers.tile([P, S], F32)          # O^T (per-head reuse)

            nc.sync.dma_start(cosT_sb[:], cosT[:])
            nc.sync.dma_start(sinT_sb[:], sinT[:])
            nc.sync.dma_start(qw_sb[:], qw[:])
            nc.sync.dma_start(kw_sb[:], kw[:])
            nc.sync.dma_start(rt_sb[:], rt[:])
            nc.sync.dma_start(ident_sb[:], ident[:])
            nc.sync.dma_start(onec_sb[:], onec[:])
            nc.sync.dma_start(oner_sb[:], oner[:])
            nc.sync.dma_start(triu_sb[:], triu[:])
            nc.gpsimd.memset(eps_sb[:], EPS)

            # mid scope: raw projection outputs that must survive into the norm
            with tc.tile_pool(name="mid", bufs=1) as midp:
                qt_raw = midp.tile([P, HPC, S], F32)
                kt_raw = midp.tile([P, S], F32)

                # ================= P1: projections =================
                with (
                    tc.tile_pool(name="wts", bufs=1) as wpool,
                    tc.tile_pool(name="hs", bufs=2) as hspool,
                    tc.tile_pool(name="vts", bufs=2) as vtpool,
                    tc.tile_pool(name="pp1", bufs=4, space="PSUM") as pp1,
                ):
                    wq_sb = wpool.tile([P, KCH, HPC * P], F32R)
                    wg_sb = wpool.tile([P, KCH, HPC * P], F32R)
                    wk_sb = wpool.tile([P, KCH, P], F32R)
                    wv_sb = wpool.tile([P, KCH, P], F32R)
                    nc.sync.dma_start(wq_sb[:], wq.rearrange("(kc p) m -> p kc m", p=P))
                    nc.sync.dma_start(wg_sb[:], wg.rearrange("(kc p) m -> p kc m", p=P))
                    nc.sync.dma_start(wk_sb[:], wk.rearrange("(kc p) m -> p kc m", p=P))
                    nc.sync.dma_start(wv_sb[:], wv.rearrange("(kc p) m -> p kc m", p=P))

                    hsTr = hsT.rearrange("(kc p) s -> p kc s", p=P)
                    SE = 256
                    for se in range(S // SE):  # s slices of 256
                        s0 = se * SE
                        hs_sb = hspool.tile([P, KCH, SE], F32)
                        for kc in range(KCH):
                            nc.sync.dma_start(
                                hs_sb[:, kc, :].bitcast(F32R),
                                hsTr[:, kc, s0:s0 + SE])
                        blocks = [(wq_sb, 0, "q0"), (wq_sb, P, "q1"),
                                  (wg_sb, 0, "g0"), (wg_sb, P, "g1"),
                                  (wk_sb, 0, "k"), (wv_sb, 0, "v")]
                        for (w_sb, coff, tag) in blocks:
                            ps = pp1.tile([P, SE], F32)
                            for kc in range(KCH):
                                nc.tensor.matmul(
                                    ps[:],
                                    lhsT=w_sb[:, kc, coff:coff + P],
                                    rhs=hs_sb[:, kc, :].bitcast(F32R),
                                    start=(kc == 0), stop=(kc == KCH - 1),
                                )
                            if tag == "q0":
                                nc.scalar.copy(qt_raw[:, 0, s0:s0 + SE], ps[:])
                            elif tag == "q1":
                                nc.scalar.copy(qt_raw[:, 1, s0:s0 + SE], ps[:])
                            elif tag == "g0":
                                nc.scalar.copy(gt_raw[:, 0, s0:s0 + SE], ps[:])
                            elif tag == "g1":
                                nc.scalar.copy(gt_raw[:, 1, s0:s0 + SE], ps[:])
                            elif tag == "k":
                                nc.scalar.copy(kt_raw[:, s0:s0 + SE], ps[:])
                            else:  # v: copy then transpose 2 blocks into v_r
                                vts = vtpool.tile([P, SE], F32R)
                                nc.scalar.copy(vts[:], ps[:])
                                for j in range(SE // P):
                                    vb = (se * SE) // P + j
                                    pst = pp1.tile([P, P], F32R, tag="pst")
                                    nc.tensor.transpose(
                                        pst[:], vts[:, j * P:(j + 1) * P], ident_sb[:])
                                    nc.scalar.copy(v_r[:, vb, :], pst[:])

                # ================= P2: norm + rope (K, then per-head Q) ===========
                def norm_rope(xsrc, wvec, xdst, npool, pps):
                    sq = npool.tile([P, S], F32R, tag="sq")
                    nc.vector.tensor_mul(sq[:], xsrc, xsrc)
                    rq = npool.tile([1, S], F32R, tag="rq")
                    sqv = npool.tile([1, S], F32, tag="sqv")
                    for sc in range(NSC):
                        sl = slice(sc * 512, (sc + 1) * 512)
                        ps1 = pps.tile([1, 512], F32, tag="ps1")
                        nc.tensor.matmul(ps1[:], lhsT=onec_sb[:], rhs=sq[:, sl],
                                         start=True, stop=True)
                        nc.scalar.activation(sqv[:, sl], ps1[:],
                                             AF.Sqrt, scale=1.0 / D, bias=eps_sb[:])
                        nc.vector.reciprocal(rq[:, sl], sqv[:, sl])
                    xn = npool.tile([P, S], F32R, tag="xn")
                    for sc in range(NSC):
                        sl = slice(sc * 512, (sc + 1) * 512)
                        psb = pps.tile([P, 512], F32, tag="psb")
                        nc.tensor.matmul(psb[:], lhsT=oner_sb[:], rhs=rq[:, sl],
                                         start=True, stop=True)
                        nc.vector.scalar_tensor_tensor(
                            xn[:, sl], xsrc[:, sl], wvec[:], psb[:],
                            op0=ALU.mult, op1=ALU.mult)
                    for sc in range(NSC):
                        sl = slice(sc * 512, (sc + 1) * 512)
                        psr = pps.tile([P, 512], F32, tag="psr")
                        nc.tensor.matmul(psr[:], lhsT=rt_sb[:], rhs=xn[:, sl],
                                         start=True, stop=True)
                        t1 = npool.tile([P, 512], F32, tag="t1", bufs=2)
                        t2 = npool.tile([P, 512], F32, tag="t2", bufs=2)
                        nc.gpsimd.tensor_mul(t1[:], xn[:, sl], cosT_sb[:, sl])
                        nc.vector.tensor_mul(t2[:], psr[:], sinT_sb[:, sl])
                        nc.vector.tensor_add(xdst[:, sl], t1[:], t2[:])

                with tc.tile_pool(name="norm", bufs=1) as npool:
                    with tc.tile_pool(name="ppsk", bufs=2, space="PSUM") as pps:
                        norm_rope(kt_raw[:], kw_sb, khat, npool, pps)

                    # per-head: Q norm+rope, attention, gating
                    for h in range(HPC):
                        with tc.tile_pool(name=f"ppsq{h}", bufs=2, space="PSUM") as pps:
                            norm_rope(qt_raw[:, h, :], qw_sb, qhat, npool, pps)

                        # ================= P3: attention =================
                        with (
                            tc.tile_pool(name="sc", bufs=4, space="PSUM") as scpool,
                            tc.tile_pool(name="pt", bufs=2, space="PSUM") as ptpool,
                            tc.tile_pool(name="ot", bufs=2, space="PSUM") as otpool,
                            tc.tile_pool(name="pu", bufs=5) as pupool,
                            tc.tile_pool(name="dd", bufs=5) as ddpool,
                            tc.tile_pool(name="sm", bufs=8) as smpool,
                            tc.tile_pool(name="pts", bufs=3) as ptspool,
                        ):
                            for qc in range(NSC):
                                pu_l = {}
                                d_l = {}
                                for qb in range(4 * qc, 4 * qc + 4):
                                    nfull = qb // 4
                                    r = qb % 4
                                    pu = pupool.tile([P, S], F32R, tag="pu")
                                    mparts = smpool.tile([P, 8], F32, tag="mp")
                                    ps_l = []
                                    for kc in range(nfull + 1):
                                        w = 512 if kc < nfull else (r + 1) * P
                                        ps = scpool.tile([P, 512], F32)
                                        nc.tensor.matmul(
                                            ps[:, :w],
                                            lhsT=qhat[:, qb * P:(qb + 1) * P],
                                            rhs=khat[:, kc * 512:kc * 512 + w],
                                            start=True, stop=True)
                                        if kc == nfull:
                                            nc.vector.tensor_add(
                                                ps[:, r * P:(r + 1) * P],
                                                ps[:, r * P:(r + 1) * P], triu_sb[:])
                                        nc.vector.tensor_reduce(
                                            mparts[:, kc:kc + 1], ps[:, :w],
                                            axis=AX, op=ALU.max)
                                        ps_l.append((ps, w))
                                    negm = smpool.tile([P, 1], F32, tag="negm")
                                    nc.vector.tensor_reduce(
                                        negm[:], mparts[:, :nfull + 1], axis=AX,
                                        op=ALU.max, negate=True)
                                    bias_t = smpool.tile([P, 1], F32, tag="bias")
                                    nc.vector.tensor_scalar_mul(bias_t[:], negm[:], SCALING)
                                    zparts = smpool.tile([P, 8], F32, tag="zp")
                                    for kc, (ps, w) in enumerate(ps_l):
                                        nc.scalar.activation(
                                            pu[:, kc * 512:kc * 512 + w], ps[:, :w],
                                            AF.Exp, scale=SCALING, bias=bias_t[:],
                                            accum_out=zparts[:, kc:kc + 1])
                                    zsum = smpool.tile([P, 1], F32, tag="zs")
                                    nc.vector.tensor_reduce(
                                        zsum[:], zparts[:, :nfull + 1], axis=AX, op=ALU.add)
                                    rz = smpool.tile([P, 1], F32, tag="rz")
                                    nc.vector.reciprocal(rz[:], zsum[:])
                                    dmat = ddpool.tile([P, P], F32R, tag="dm")
                                    nc.vector.tensor_scalar_mul(dmat[:], ident_sb[:], rz[:])
                                    pu_l[qb] = pu
                                    d_l[qb] = dmat
                                # PuT (normalized) + AV accumulation for this q-chunk
                                ot_ps = otpool.tile([P, 512], F32)
                                kmax = 4 * qc + 3
                                for kb in range(kmax + 1):
                                    putp = ptpool.tile([P, 512], F32)
                                    i0 = max(kb - 4 * qc, 0)
                                    if i0 > 0:
                                        nc.vector.memset(putp[:, :i0 * P], 0.0)
                                    for j in range(4):
                                        qb = 4 * qc + j
                                        if kb > qb:
                                            continue
                                        nc.tensor.matmul(
                                            putp[:, j * P:(j + 1) * P],
                                            lhsT=pu_l[qb][:, kb * P:(kb + 1) * P],
                                            rhs=d_l[qb][:],
                                            start=True, stop=True)
                                    puts = ptspool.tile([P, 512], F32R)
                                    nc.scalar.copy(puts[:], putp[:])
                                    nc.tensor.matmul(
                                        ot_ps[:], lhsT=v_r[:, kb, :], rhs=puts[:],
                                        start=(kb == 0), stop=(kb == kmax))
                                nc.scalar.copy(ot_sb[:, qc * 512:(qc + 1) * 512], ot_ps[:])

                        # gating for this head (sigmoid table set swap happens here)
                        for sc in range(NSC):
                            sl = slice(sc * 512, (sc + 1) * 512)
                            sig = npool.tile([P, 512], F32, tag="sig", bufs=2)
                            nc.scalar.activation(sig[:], gt_raw[:, h, sl], AF.Sigmoid)
                            nc.vector.tensor_mul(gated_r[:, h, sl], ot_sb[:, sl], sig[:])

            # ================= P4: Wo =================
            with (
                tc.tile_pool(name="wop", bufs=1) as wopool,
                tc.tile_pool(name="po", bufs=4, space="PSUM") as popool,
                tc.tile_pool(name="co", bufs=4) as copool,
            ):
                wo_sb = wopool.tile([P, HPC, DM], F32R)
                nc.sync.dma_start(wo_sb[:], wo.rearrange("(h p) m -> p h m", p=P))
                for sb in range(NQB):
                    for dc in range(NSC):
                        pso = popool.tile([P, 512], F32)
                        for h in range(HPC):
                            nc.tensor.matmul(
                                pso[:],
                                lhsT=gated_r[:, h, sb * P:(sb + 1) * P],
                                rhs=wo_sb[:, h, dc * 512:(dc + 1) * 512],
                                start=(h == 0), stop=(h == HPC - 1))
                        cpo = copool.tile([P, 512], F32)
                        nc.scalar.copy(cpo[:], pso[:])
                        nc.sync.dma_start(
                            out[sb * P:(sb + 1) * P, dc * 512:(dc + 1) * 512], cpo[:])

    nc.compile()
    return nc


def _host_inputs(hidden_states, cos, sin, Wq, Wk, Wv, Wo, q_norm_w, k_norm_w):
    hs = np.asarray(hidden_states, dtype=np.float32).reshape(S, DM)
    hsT_r = _round_fp32r(np.ascontiguousarray(hs.T))
    cosT = np.ascontiguousarray(np.asarray(cos, np.float32).T)
    sinT = np.ascontiguousarray(np.asarray(sin, np.float32).T)
    rtm = np.zeros((P, P), np.float32)
    half = D // 2
    for d in range(half):          # RT[d', d] such that (RT.T @ x)[d] = rot(x)[d]
        rtm[d + half, d] = -1.0    # rot[d<64] = -x[d+64]
    for d in range(half, D):
        rtm[d - half, d] = 1.0     # rot[d>=64] = +x[d-64]
    ident = np.eye(P, dtype=np.float32)
    onec = np.ones((P, 1), np.float32)
    oner = np.ones((1, P), np.float32)
    triu = np.triu(np.full((P, P), -1e9, np.float32), 1)
    Wq = np.asarray(Wq, np.float32)
    Wk = np.asarray(Wk, np.float32)
    Wv = np.asarray(Wv, np.float32)
    Wo = np.asarray(Wo, np.float32)
    maps = []
    for c in range(NCORES):
        heads = [2 * c, 2 * c + 1]
        g = c // 2
        wq_c = np.concatenate([Wq[:, h * 2 * D:h * 2 * D + D] for h in heads], axis=1)
        wg_c = np.concatenate([Wq[:, h * 2 * D + D:(h + 1) * 2 * D] for h in heads], axis=1)
        maps.append({
            "hsT": hsT_r,
            "wq": _round_fp32r(wq_c),
            "wg": _round_fp32r(wg_c),
            "wk": _round_fp32r(Wk[:, g * D:(g + 1) * D]),
            "wv": _round_fp32r(Wv[:, g * D:(g + 1) * D]),
            "wo": _round_fp32r(Wo[c * 2 * D:(c + 1) * 2 * D, :]),
            "cosT": cosT, "sinT": sinT,
            "qw": np.asarray(q_norm_w, np.float32).reshape(P, 1),
            "kw": np.asarray(k_norm_w, np.float32).reshape(P, 1),
            "rt": rtm, "ident": ident, "onec": onec, "oner": oner, "triu": triu,
        })
    return maps


def kernel(**inputs):
    from concourse.bass_utils import run_bass_kernel_spmd

    if "nc" not in _cache:
        _cache["nc"] = _build_nc()
    nc = _cache["nc"]
    maps = _host_inputs(
        inputs["hidden_states"], inputs["cos"], inputs["sin"],
        inputs["Wq"], inputs["Wk"], inputs["Wv"], inputs["Wo"],
        inputs["q_norm_w"], inputs["k_norm_w"])
    res = run_bass_kernel_spmd(nc, maps, list(range(NCORES)))
    total = np.zeros((S, DM), np.float64)
    for r in res.results:
        total += r["out"].astype(np.float64)
    return total.astype(np.float32).reshape(1, S, DM)
